# revision 7
# baseline (speedup 1.0000x reference)
"""Trainium2 Bass kernel for the DigitConvolutionalModel problem.

Math: out = relu(conv3x3(x) @ fc1_w.T + fc1_b) @ fc2_w.T + fc2_b
The 3x3 valid conv followed by a dense layer composes into a single
linear map, so conv_w and fc1_w are folded on the host into one
W1eff [128, 784] matrix. The device then runs two matmuls + bias/relu.

Sharding: pure data parallelism - batch split across 8 cores.
Each core's x shard is staged transposed ([784, 8192]) so the
contraction dim lands on SBUF partitions with contiguous DMA.

Precision: plain fp16 for x and all weights with f32 PSUM
accumulation. The correctness budget (rel err 2e-2) dwarfs fp16
matmul error (~1e-3 here), and fp16 halves HBM traffic vs f32 or a
compensated hi+lo fp16 pair. The kernel is DMA-bound: ~12.9 MB of x
per core against a ~360 GB/s modeled bus. K=784 is split uniformly
as 7 chunks x 112 partitions (no ragged 16-row tail pass).
"""

import numpy as np

import concourse.bacc as bacc
import concourse.mybir as mybir
import concourse.tile as tile
from concourse.bass_utils import run_bass_kernel_spmd

N_CORES = 8
B = 65536
B_LOCAL = B // N_CORES  # 8192
K = 784                 # input features (28*28)
KP = 112                # K rows per chunk (7 * 112 = 784)
NKC = 7                 # K chunks
M1 = 128                # fc1 out
M2 = 10                 # fc2 out

F32 = mybir.dt.float32
FP16 = mybir.dt.float16

NS = 512                # matmul moving-dim subtile (one PSUM bank)

# Batch-tile schedule: small tiles at the edges so the PE starts early
# (pipeline fill) and finishes right behind the last transfer (drain);
# big tiles in the middle keep the SWDGE generation queue off the
# critical path.
BTS = [256, 256, 512, 1024, 1024, 1024, 1024, 1024, 1024, 512, 256, 256]
assert sum(BTS) == B_LOCAL

_cache = {}


def _build_nc():
    nc = bacc.Bacc("TRN2", target_bir_lowering=False, debug=False,
                   num_devices=N_CORES)

    x_d = nc.dram_tensor("x_t", [K, B_LOCAL], FP16, kind="ExternalInput")
    w1_d = nc.dram_tensor("w1_t", [KP, NKC, M1], FP16, kind="ExternalInput")
    w2_d = nc.dram_tensor("w2_t", [M1, M2], FP16, kind="ExternalInput")
    # f32 pack: col 0 = b1, col 1 rows 0:10 = b2
    bias_d = nc.dram_tensor("biases", [M1, 2], F32, kind="ExternalInput")
    z_d = nc.dram_tensor("z_t", [M2, B_LOCAL], FP16, kind="ExternalOutput")

    x_v = x_d.rearrange("(c p) b -> p c b", p=KP)

    with tile.TileContext(nc) as tc:
        with (
            tc.tile_pool(name="static", bufs=1) as sp,
            tc.tile_pool(name="xp", bufs=4) as xp,
            tc.tile_pool(name="hp", bufs=4) as hp,
            tc.tile_pool(name="zp", bufs=3) as zp,
            tc.tile_pool(name="pp1", bufs=4, space="PSUM") as pp1,
            tc.tile_pool(name="pp2", bufs=2, space="PSUM") as pp2,
        ):
            # Weights ride the SWDGE queue ahead of x tile 0; the small
            # fc2/bias tensors take the (otherwise idle) SP HWDGE path.
            w1 = sp.tile([KP, NKC, M1], FP16, tag="w1")
            nc.gpsimd.dma_start(w1[:], w1_d[:])
            w2 = sp.tile([M1, M2], FP16, tag="w2")
            nc.sync.dma_start(w2[:], w2_d[:])
            bias = sp.tile([M1, 2], F32, tag="biases")
            nc.sync.dma_start(bias[:], bias_d[:])
            b1t = bias[:, 0:1]
            b2t = bias[0:M2, 1:2]

            offs = [sum(BTS[:i]) for i in range(len(BTS))]
            xtiles = [None] * len(BTS)

            def load_bt(i):
                btc = BTS[i]
                xt = xp.tile([KP, NKC, btc], FP16, tag="x")
                nc.gpsimd.dma_start(
                    xt[:], x_v[:, :, offs[i]:offs[i] + btc])
                xtiles[i] = xt

            for i in range(5):
                load_bt(i)

            # Each chunk's fc2 matmul + bias-add is deferred until after
            # the NEXT chunk's fc1 stream, so the PE never sits behind an
            # fc2 that is still waiting on ACT's h output.
            pending = []

            def flush_pending():
                for h_t, zt_t, sl_t, final in pending:
                    ps2 = pp2.tile([M2, sl_t.stop - sl_t.start], F32,
                                   tag="ps2")
                    nc.tensor.matmul(ps2[:], w2[:], h_t[:],
                                     start=True, stop=True)
                    nc.vector.tensor_scalar_add(zt_t[:, sl_t], ps2[:], b2t)
                    if final is not None:
                        nc.sync.dma_start(final[0], zt_t[:])
                pending.clear()

            for bt_i, btc in enumerate(BTS):
                if bt_i + 5 < len(BTS):
                    load_bt(bt_i + 5)
                xt = xtiles[bt_i]
                zt = zp.tile([M2, btc], FP16, tag="z")
                ns = min(NS, btc)
                nchains = btc // ns
                for ns_i in range(nchains):
                    sl = slice(ns_i * ns, (ns_i + 1) * ns)
                    ps1 = pp1.tile([M1, ns], F32, tag="ps1")
                    for c in range(NKC):
                        nc.tensor.matmul(
                            ps1[:], w1[:, c, :], xt[:, c, sl],
                            start=(c == 0), stop=(c == NKC - 1))
                    h = hp.tile([M1, ns], FP16, tag="h")
                    nc.scalar.activation(
                        h[:], ps1[:], mybir.ActivationFunctionType.Relu,
                        bias=b1t)
                    flush_pending()
                    final = None
                    if ns_i == nchains - 1:
                        final = (z_d[:, offs[bt_i]:offs[bt_i] + btc],)
                    pending.append((h, zt, sl, final))
                    if bt_i >= len(BTS) - 3:
                        # Drain region: x is nearly exhausted, so the PE
                        # has slack - flush immediately to shorten the
                        # ACT->fc2->DVE->DMA tail cascade.
                        flush_pending()
            flush_pending()
    nc.compile()
    return nc


def _fold_weights(conv_w, fc1_w):
    """Fold 3x3 valid cross-correlation + fc1 into one [128, 784] matrix."""
    cw = np.asarray(conv_w, np.float64)
    f1 = np.asarray(fc1_w, np.float64).reshape(M1, 26, 26)
    W = np.zeros((M1, 28, 28), np.float64)
    for di in range(3):
        for dj in range(3):
            W[:, di:di + 26, dj:dj + 26] += cw[di, dj] * f1
    return W.reshape(M1, K).astype(np.float32)


def kernel(x, conv_w, fc1_w, fc1_b, fc2_w, fc2_b):
    if "nc" not in _cache:
        _cache["nc"] = _build_nc()
    nc = _cache["nc"]

    w1t = _fold_weights(conv_w, fc1_w).T.astype(np.float16)  # [784, 128]
    w1t = np.ascontiguousarray(
        w1t.reshape(NKC, KP, M1).transpose(1, 0, 2))  # [112, 7, 128]
    w2t = np.ascontiguousarray(
        np.asarray(fc2_w, np.float16).T)  # [128, 10]
    biases = np.zeros((M1, 2), np.float32)
    biases[:, 0] = np.asarray(fc1_b, np.float32)
    biases[0:M2, 1] = np.asarray(fc2_b, np.float32)
    x = np.asarray(x, np.float32)

    in_maps = []
    for c in range(N_CORES):
        xs = np.ascontiguousarray(
            x[c * B_LOCAL:(c + 1) * B_LOCAL].T.astype(np.float16))
        in_maps.append({
            "x_t": xs, "w1_t": w1t, "w2_t": w2t, "biases": biases,
        })
    res = run_bass_kernel_spmd(nc, in_maps, list(range(N_CORES)))
    outs = [res.results[c]["z_t"].T for c in range(N_CORES)]
    return np.ascontiguousarray(np.concatenate(outs, axis=0), dtype=np.float32)
